# revision 1
# baseline (speedup 1.0000x reference)
"""LoRA Linear layer on 8 Trainium2 NeuronCores.

Computes out = x @ W.T + bias + scaling * (x @ A.T) @ B.T for
x [4, 4096, 4096] f32, W [4096, 4096], bias [4096], A [16, 4096], B [4096, 16].

Strategy:
- Host: fold the rank-16 LoRA path into the weight (exact up to f32
  rounding): W_eff = W.T + scaling * (A.T @ B.T), layout [in, out].
- Shard data-parallel over the batch: 16384 rows of x split 8 x 2048.
  W_eff/bias replicated per core; no collectives.
- Per core: out_s[2048, 4096] = x_s @ W_eff + bias as an fp16 matmul with
  fp32 PSUM accumulation (scale-relative absmax error ~3e-4 vs f32).
- PE structure: x m-tile [128,128] is the stationary operand, reused for
  2 consecutive matmuls (2 n-tiles of 512) — the implicit LDWEIGHTS is
  skipped when the weights AP repeats (measured 136ns/MM same-stationary
  vs 222ns/MM with a fresh stationary per matmul).
- SBUF: W n-blocks of [4096, 1024] fp16 (32 chunks of [128,1024], 64KB/
  partition) double-buffered so the next block streams during the
  current block's compute; x streams per-m-tile as packed [128,32,128]
  chunks (host pre-packs for contiguous DMA). DMA streams use separate
  engines (W: sync/HWDGE, x: gpsimd, out: vector) so slot-wait
  head-of-line blocking can't cross streams.
"""

import numpy as np

IN_F = 4096
OUT_F = 4096
R = 16
SCALING = 32.0 / R
N_CORES = 8
M_TOTAL = 4 * 4096
M_CORE = M_TOTAL // N_CORES  # 2048

P = 128
KO = IN_F // P  # 32 contraction chunks
NW = 512  # matmul free dim (one PSUM bank of f32)
NJ = 2  # n-tiles per block (stationary reused NJ times)
NB = OUT_F // (NJ * NW)  # 4 n blocks
NBW = NJ * NW  # 1024 cols per block
MT = M_CORE // P  # 16 m tiles

_CACHE = {}


def _build_nc(repeats=1, fake_w=False):
    """repeats>1 replays the whole compute pass (W/x re-streamed) — used
    only for device-time measurement by test.py. fake_w=True reuses one
    W block everywhere (numerically wrong; isolates W-DMA stalls)."""
    import concourse.mybir as mybir
    import concourse.tile as tile
    from concourse import bacc

    nc = bacc.Bacc("TRN2", target_bir_lowering=False, debug=False,
                   num_devices=N_CORES)
    xk = nc.dram_tensor("xk", [MT, P, KO, P], mybir.dt.float16,
                        kind="ExternalInput").ap()
    w = nc.dram_tensor("w", [IN_F, OUT_F], mybir.dt.float16,
                       kind="ExternalInput").ap()
    biasr = nc.dram_tensor("biasr", [P, OUT_F], mybir.dt.float32,
                           kind="ExternalInput").ap()
    out = nc.dram_tensor("out", [M_CORE, OUT_F], mybir.dt.float32,
                         kind="ExternalOutput").ap()

    wr = w.rearrange("(ko p) n -> ko p n", p=P)

    # (rep, nb) blocks in execution order
    blocks = [(rep, nb) for rep in range(repeats) for nb in range(NB)]

    with tile.TileContext(nc) as tc:
        with (
            tc.tile_pool(name="xpool", bufs=3) as xpool,
            tc.tile_pool(name="wpool", bufs=66) as wpool,
            tc.tile_pool(name="bpool", bufs=1) as bpool,
            tc.tile_pool(name="opool", bufs=4) as opool,
            tc.tile_pool(name="pspool", bufs=8, space="PSUM") as pspool,
        ):
            bias_sb = bpool.tile([P, OUT_F], mybir.dt.float32, name="bias_sb")
            nc.gpsimd.dma_start(bias_sb[:], biasr)

            w_sb = {}

            def load_w_block(bi):
                rep, nb = blocks[bi]
                if fake_w and bi > 0:
                    w_sb[nb] = w_sb[blocks[0][1]]
                    return
                for ko in range(KO):
                    wt = wpool.tile([P, NBW], mybir.dt.float16,
                                    name=f"w{rep}_{nb}_{ko}", tag="w",
                                    bufs=66)
                    nc.sync.dma_start(
                        wt[:], wr[ko, :, nb * NBW:(nb + 1) * NBW])
                    w_sb.setdefault(nb, [None] * KO)
                    w_sb[nb] = (w_sb[nb] if len(w_sb[nb]) == KO else w_sb[nb])
                    w_sb[nb][ko] = wt

            # preload the first two blocks' W (double buffer warm)
            load_w_block(0)
            if len(blocks) > 1:
                load_w_block(1)

            for bi, (rep, nb) in enumerate(blocks):
                wts = w_sb[nb]
                for mt in range(MT):
                    xm = xpool.tile([P, KO, P], mybir.dt.float16,
                                    name=f"xm{rep}_{nb}_{mt}", tag="x")
                    nc.gpsimd.dma_start(xm[:], xk[mt])
                    psums = [
                        pspool.tile([P, NW], mybir.dt.float32,
                                    name=f"ps_{rep}_{nb}_{mt}_{nj}",
                                    tag="ps")
                        for nj in range(NJ)
                    ]
                    for ko in range(KO):
                        lhsT = xm[:, ko, :]
                        wt = wts[ko]
                        for nj in range(NJ):
                            nc.tensor.matmul(
                                psums[nj][:],
                                lhsT,
                                wt[:, nj * NW:(nj + 1) * NW],
                                start=(ko == 0),
                                stop=(ko == KO - 1),
                            )
                    m0 = mt * P
                    for nj in range(NJ):
                        c0 = nb * NBW + nj * NW
                        ot = opool.tile([P, NW], mybir.dt.float32,
                                        name=f"o_{rep}_{nb}_{mt}_{nj}",
                                        tag="o")
                        nc.vector.tensor_add(
                            ot[:], psums[nj][:], bias_sb[:, c0:c0 + NW])
                        nc.scalar.dma_start(
                            out[m0:m0 + P, c0:c0 + NW], ot[:])

                    # kick off block bi+2's W stream near the start of this
                    # block (its slots are free: block bi-1 fully consumed)
                    if mt == 1 and bi + 2 < len(blocks):
                        load_w_block(bi + 2)

    nc.compile()
    return nc


def _get_nc():
    if "nc" not in _CACHE:
        _CACHE["nc"] = _build_nc()
    return _CACHE["nc"]


def make_in_maps(x, weight, bias, lora_A, lora_B):
    """Host-side shard prep: returns the per-core input maps."""
    w_eff = weight.T.astype(np.float32) + np.float32(SCALING) * (
        lora_A.T.astype(np.float32) @ lora_B.T.astype(np.float32))
    w16 = w_eff.astype(np.float16)
    biasr = np.ascontiguousarray(
        np.broadcast_to(bias.astype(np.float32), (P, OUT_F)))
    xf = np.asarray(x, dtype=np.float32).reshape(M_TOTAL, IN_F)
    in_maps = []
    for c in range(N_CORES):
        xs = xf[c * M_CORE:(c + 1) * M_CORE]
        xT = np.ascontiguousarray(xs.T, dtype=np.float16)  # [IN_F, M_CORE]
        # pack to [m_tile, p, ko, m] so each m-tile is one contiguous DMA
        xk = np.ascontiguousarray(
            xT.reshape(KO, P, MT, P).transpose(2, 1, 0, 3))
        in_maps.append({"xk": xk, "w": w16, "biasr": biasr})
    return in_maps


def kernel(x, weight, bias, lora_A, lora_B):
    from concourse.bass_utils import run_bass_kernel_spmd

    nc = _get_nc()
    in_maps = make_in_maps(x, weight, bias, lora_A, lora_B)
    res = run_bass_kernel_spmd(nc, in_maps, core_ids=list(range(N_CORES)))
    _CACHE["last_result"] = res
    out = np.concatenate([r["out"] for r in res.results], axis=0)
    return out.reshape(4, 4096, OUT_F)



# revision 2
# speedup vs baseline: 2.7306x; 2.7306x over previous
"""LoRA Linear layer on 8 Trainium2 NeuronCores.

Computes out = x @ W.T + bias + scaling * (x @ A.T) @ B.T for
x [4, 4096, 4096] f32, W [4096, 4096], bias [4096], A [16, 4096], B [4096, 16].

Strategy:
- Host: fold the rank-16 LoRA path into the weight (exact up to f32
  rounding): W_eff = W.T + scaling * (A.T @ B.T), layout [in, out].
- Shard data-parallel over the batch: 16384 rows of x split 8 x 2048.
  W_eff/bias replicated per core; no collectives.
- Per core: out_s[2048, 4096] = x_s @ W_eff + bias as an fp16 matmul with
  fp32 PSUM accumulation (scale-relative absmax error ~3e-4 vs f32).
- PE structure: x m-tile [128,128] is the stationary operand, reused for
  2 consecutive matmuls (2 n-tiles of 512) — the implicit LDWEIGHTS is
  skipped when the weights AP repeats. Microbenched same-window: this
  2x-reuse pattern issues N=512 fp16 matmuls at ~204-240 ns/MM — the
  1-col/cycle PE streaming roofline. (4x/8x stationary reuse measured
  25% SLOWER per MM; W-stationary/x-resident restructure measured ~5-10%
  slower end-to-end; fp8 would be ~1.5x faster but its ~3.7% quantization
  error fails the 2e-2 gate.)
- The kernel is PE-bound: removing the W stream, the x stream, or the
  out stream (fake-DMA variants) does not change the pass time, so all
  HBM traffic is fully hidden behind compute.
- SBUF: W n-blocks of [4096, 1024] fp16 (32 chunks of [128,1024], 64KB/
  partition) double-buffered; block bi+2's chunk stream is spread evenly
  across block bi's m-tiles (2 chunks per m-tile) instead of bursting.
  x streams per-m-tile as packed [128,32,128] chunks (host pre-packs for
  contiguous DMA), 5-deep prefetch ring. DMA streams use separate
  queues (W: sync/HWDGE, x: gpsimd/SWDGE, out: scalar/HWDGE) so
  slot-wait head-of-line blocking can't cross streams.
"""

import numpy as np

IN_F = 4096
OUT_F = 4096
R = 16
SCALING = 32.0 / R
N_CORES = 8
M_TOTAL = 4 * 4096
M_CORE = M_TOTAL // N_CORES  # 2048

P = 128
KO = IN_F // P  # 32 contraction chunks
NW = 512  # matmul free dim (one PSUM bank of f32)
NJ = 2  # n-tiles per block (stationary reused NJ times)
NB = OUT_F // (NJ * NW)  # 4 n blocks
NBW = NJ * NW  # 1024 cols per block
MT = M_CORE // P  # 16 m tiles

_CACHE = {}


def _build_nc(repeats=1, fake_w=False):
    """repeats>1 replays the whole compute pass (W/x re-streamed) — used
    only for device-time measurement by test.py. fake_w=True reuses one
    W block everywhere (numerically wrong; isolates W-DMA stalls)."""
    import concourse.mybir as mybir
    import concourse.tile as tile
    from concourse import bacc

    nc = bacc.Bacc("TRN2", target_bir_lowering=False, debug=False,
                   num_devices=N_CORES)
    xk = nc.dram_tensor("xk", [MT, P, KO, P], mybir.dt.float16,
                        kind="ExternalInput").ap()
    w = nc.dram_tensor("w", [IN_F, OUT_F], mybir.dt.float16,
                       kind="ExternalInput").ap()
    biasr = nc.dram_tensor("biasr", [P, OUT_F], mybir.dt.float32,
                           kind="ExternalInput").ap()
    out = nc.dram_tensor("out", [M_CORE, OUT_F], mybir.dt.float32,
                         kind="ExternalOutput").ap()

    wr = w.rearrange("(ko p) n -> ko p n", p=P)

    # (rep, nb) blocks in execution order
    blocks = [(rep, nb) for rep in range(repeats) for nb in range(NB)]

    with tile.TileContext(nc) as tc:
        with (
            tc.tile_pool(name="xpool", bufs=5) as xpool,
            tc.tile_pool(name="wpool", bufs=66) as wpool,
            tc.tile_pool(name="bpool", bufs=1) as bpool,
            tc.tile_pool(name="opool", bufs=4) as opool,
            tc.tile_pool(name="pspool", bufs=8, space="PSUM") as pspool,
        ):
            bias_sb = bpool.tile([P, OUT_F], mybir.dt.float32, name="bias_sb")
            nc.gpsimd.dma_start(bias_sb[:], biasr)

            w_sb = {}

            def load_w_chunks(bi, kos):
                rep, nb = blocks[bi]
                if fake_w and bi > 0:
                    w_sb[nb] = w_sb[blocks[0][1]]
                    return
                for ko in kos:
                    wt = wpool.tile([P, NBW], mybir.dt.float16,
                                    name=f"w{rep}_{nb}_{ko}", tag="w",
                                    bufs=66)
                    nc.sync.dma_start(
                        wt[:], wr[ko, :, nb * NBW:(nb + 1) * NBW])
                    w_sb.setdefault(nb, [None] * KO)
                    w_sb[nb][ko] = wt

            # preload the first two blocks' W (double buffer warm)
            load_w_chunks(0, range(KO))
            if len(blocks) > 1:
                load_w_chunks(1, range(KO))

            for bi, (rep, nb) in enumerate(blocks):
                wts = w_sb[nb]
                for mt in range(MT):
                    xm = xpool.tile([P, KO, P], mybir.dt.float16,
                                    name=f"xm{rep}_{nb}_{mt}", tag="x")
                    nc.gpsimd.dma_start(xm[:], xk[mt])
                    psums = [
                        pspool.tile([P, NW], mybir.dt.float32,
                                    name=f"ps_{rep}_{nb}_{mt}_{nj}",
                                    tag="ps")
                        for nj in range(NJ)
                    ]
                    for ko in range(KO):
                        lhsT = xm[:, ko, :]
                        wt = wts[ko]
                        for nj in range(NJ):
                            nc.tensor.matmul(
                                psums[nj][:],
                                lhsT,
                                wt[:, nj * NW:(nj + 1) * NW],
                                start=(ko == 0),
                                stop=(ko == KO - 1),
                            )
                        # spread block bi+2's W stream: 2 chunks per m-tile
                        # instead of a 8.4 MB burst at mt==1
                        if ko % 16 == 0 and bi + 2 < len(blocks):
                            k2 = mt * 2 + ko // 16
                            if k2 < KO:
                                load_w_chunks(bi + 2, [k2])
                    m0 = mt * P
                    for nj in range(NJ):
                        c0 = nb * NBW + nj * NW
                        ot = opool.tile([P, NW], mybir.dt.float32,
                                        name=f"o_{rep}_{nb}_{mt}_{nj}",
                                        tag="o")
                        nc.vector.tensor_add(
                            ot[:], psums[nj][:], bias_sb[:, c0:c0 + NW])
                        nc.scalar.dma_start(
                            out[m0:m0 + P, c0:c0 + NW], ot[:])



    nc.compile()
    return nc


def _get_nc():
    if "nc" not in _CACHE:
        _CACHE["nc"] = _build_nc()
    return _CACHE["nc"]


def make_in_maps(x, weight, bias, lora_A, lora_B):
    """Host-side shard prep: returns the per-core input maps."""
    w_eff = weight.T.astype(np.float32) + np.float32(SCALING) * (
        lora_A.T.astype(np.float32) @ lora_B.T.astype(np.float32))
    w16 = w_eff.astype(np.float16)
    biasr = np.ascontiguousarray(
        np.broadcast_to(bias.astype(np.float32), (P, OUT_F)))
    xf = np.asarray(x, dtype=np.float32).reshape(M_TOTAL, IN_F)
    in_maps = []
    for c in range(N_CORES):
        xs = xf[c * M_CORE:(c + 1) * M_CORE]
        xT = np.ascontiguousarray(xs.T, dtype=np.float16)  # [IN_F, M_CORE]
        # pack to [m_tile, p, ko, m] so each m-tile is one contiguous DMA
        xk = np.ascontiguousarray(
            xT.reshape(KO, P, MT, P).transpose(2, 1, 0, 3))
        in_maps.append({"xk": xk, "w": w16, "biasr": biasr})
    return in_maps


def kernel(x, weight, bias, lora_A, lora_B):
    from concourse.bass_utils import run_bass_kernel_spmd

    nc = _get_nc()
    in_maps = make_in_maps(x, weight, bias, lora_A, lora_B)
    res = run_bass_kernel_spmd(nc, in_maps, core_ids=list(range(N_CORES)))
    _CACHE["last_result"] = res
    out = np.concatenate([r["out"] for r in res.results], axis=0)
    return out.reshape(4, 4096, OUT_F)

